# revision 77
# baseline (speedup 1.0000x reference)
"""Causal self-attention (GQA + RoPE) Trainium2 kernel, 8-way sharded.

Sharding: DP=4 over batch x TP=2 over kv-head groups (2 kv heads + their
8 q heads per group).  Each core computes its batch's qkv projection for
its head group, causal attention, and a partial c_proj (columns of
w_proj for its head group).  Host sums the two partial c_proj outputs
per batch.

Everything on-chip runs transposed ([feature, token] layout) so matmuls
contract along partitions; host transposes inputs/outputs.

Projection matmuls (qkv, v, c_proj) run as fp8e4 DoubleRow "triple-MMs":
each operand A is sent as A_hi + A_lo (both e4m3; hi = rounded value,
lo = rounded residual), and each pair of 128-deep contraction chunks is
computed with three DoubleRow matmuls
    hi*hi + hi*lo + lo*hi          (lo*lo ~ 0.06% -- dropped)
at half-rate each, i.e. 0.75x the bf16 cost with ~bf16 accuracy.
Weights are prescaled by 64 so their sigma ~ 1/45 lands in e4m3 normal
range; the inverse scale is folded into the RoPE trig tables / the
PSUM->SBUF copies.  Attention itself (QK, AV, softmax) stays bf16.

Pipeline: the attention inner loop leaves PE slack while ACT churns
exps, so the q/k projection work for head h+1 is interleaved into the
PE stream of head h's attention (weights for head h+2 prefetched
mid-stream; the stream finishes by mid strip 3 so its rope chain never
bunches at the head boundary); head 7 interleaves c_proj instead, with
its first two strips zipped k-tile-wise (they have no c work yet and
would otherwise run at ACT's exp pace).  The A0 phase runs the three
initial m-streams chunk-major (it is DMA-paced, so each arriving x
chunk is burned three times) and defers each strip's v-projection one
strip so the wv loads hide.  The AV matmuls for tile kt are emitted
four k-tiles late (lag-4 software pipeline) so the exp -> mask chain
never stalls the PE.  Causal masking only touches the one 128x128
triangle block per diagonal tile (GPSIMD for heads 0-6; DVE on head 7,
where gpsimd outT store generation would queue ahead of the masks);
the valid column range of a diagonal tile is computed mask-free.
outT stores alternate gpsimd/SP issue so no single sequencer paces the
c_proj drain.

RoPE: w_attn q/k rows are permuted per-head to [even dims; odd dims] so
rotation pairs land at partition f and f+64 of the qkv psum tile:
  P  = ps * [c; c] (SBUF),  P2 = ps * [s; s] (PSUM)
  out[0:64]   = P[0:64]  - P2[64:128]
  out[64:128] = P2[0:64] + P[64:128]
(each combine reads one SBUF + one PSUM operand, which may sit at
different base partitions; two SBUF operands may not).

Softmax: att^T tiles ([k, q] layout) are exp'd on ACT without
max-subtraction (logits are O(6), fp32-safe).  Denominator: ALL e-tiles
of a strip are tree-summed on DVE/GPSIMD (groups of 4 for full tiles;
diagonal tiles folded in over their valid column ranges only) into one
fp16 accumulator, which gets a single ones-MATRIX matmul per strip.
That matmul + the reciprocal/normalize ("finalize") are deferred into
the NEXT strip's PE stream so the sum chain never stalls the PE.  The
ones-matmul's [128, 512] output IS the denominator broadcast down all
partitions -- the per-q reciprocal is then a single DVE op.
"""

import math

import numpy as np
import ml_dtypes

import concourse.bass as bass
import concourse.mybir as mybir
import concourse.tile as tile
from concourse import bacc
from concourse.bass_utils import run_bass_kernel_spmd

ALU = mybir.AluOpType
AF = mybir.ActivationFunctionType
F32 = mybir.dt.float32
F32R = mybir.dt.float32r
BF16 = mybir.dt.bfloat16
FP16 = mybir.dt.float16
FP8 = mybir.dt.float8e4
DR = mybir.MatmulPerfMode.DoubleRow
BF = ml_dtypes.bfloat16
E4 = ml_dtypes.float8_e4m3

# problem shape (hardcoded per contest rules)
B, T, C = 4, 2048, 2048
N_HEAD, N_KV_HEAD, HD = 16, 4, 128
ROPE_THETA = 10000.0

TP = 2            # head-group shards
DP = 4            # batch shards
HQ = N_HEAD // TP         # 8 q heads per core
HKV = N_KV_HEAD // TP     # 2 kv heads per core
NREP = N_HEAD // N_KV_HEAD  # 4
QK_ROWS = (HQ + HKV) * HD   # 1280
KC = C // 128     # 16 contraction tiles
NQ = T // 512     # 4 token strips
MQK = QK_ROWS // 128  # 10 feature tiles (8 q heads + 2 kv heads)
FM = C // 128     # 16 output feature tiles
SCALE = 1.0 / math.sqrt(HD)
WS = 64.0         # weight prescale for e4m3 range

N_CORES = 8

_NC = None        # cached compiled Bass module
LAST_RUN = None   # BassKernelResults of the most recent kernel() call


def build_nc():
    nc = bacc.Bacc(None, target_bir_lowering=False, debug=False)

    xhi = nc.declare_dram_parameter("xhi", [128, KC, T], FP8, isOutput=False)
    xlo = nc.declare_dram_parameter("xlo", [128, KC, T], FP8, isOutput=False)
    wqk_hl = nc.declare_dram_parameter("wqk_hl", [MQK, 128, 2 * KC * 128], FP8, isOutput=False)
    wv_hi = nc.declare_dram_parameter("wv_hi", [128, KC * HKV * HD], FP8, isOutput=False)
    wv_lo = nc.declare_dram_parameter("wv_lo", [128, KC * HKV * HD], FP8, isOutput=False)
    wp_hl = nc.declare_dram_parameter("wp_hl", [FM, 128, 2 * HQ * 128], FP8, isOutput=False)
    trigf = nc.declare_dram_parameter("trigf", [128, T], BF16, isOutput=False)  # [c;c]/WS
    trigw = nc.declare_dram_parameter("trigw", [128, T], BF16, isOutput=False)  # [s;s]/WS
    maskd = nc.declare_dram_parameter("maskd", [128, 128], BF16, isOutput=False)
    outT = nc.declare_dram_parameter("outT", [C, T], BF16, isOutput=True)

    with tile.TileContext(nc) as tc:
        with (
            tc.tile_pool(name="const", bufs=1) as const,
            tc.tile_pool(name="persist", bufs=1) as persist,
            tc.tile_pool(name="eb", bufs=12) as eb,
            tc.tile_pool(name="gag", bufs=2) as gag,
            tc.tile_pool(name="accp", bufs=2) as accp,
            tc.tile_pool(name="rb", bufs=1) as rb,
            tc.tile_pool(name="ytp", bufs=3) as ytp,
            tc.tile_pool(name="psS", bufs=2, space="PSUM") as psS,
            tc.tile_pool(name="psY", bufs=2, space="PSUM") as psY,
            tc.tile_pool(name="psD", bufs=1, space="PSUM") as psD,
        ):
            trigf_sb = const.tile([128, T], BF16, name="trigf")
            trigw_sb = const.tile([128, T], BF16, name="trigw")
            mask_sb = const.tile([128, 128], BF16, name="mask")
            ones_mat = const.tile([128, 128], FP16, name="onem")

            qrot = [persist.tile([128, T], BF16, name=f"qrot{h}") for h in range(HQ)]
            krot = [persist.tile([128, T], BF16, name=f"krot{h}") for h in range(HKV)]
            v_sb = persist.tile([128, T // 128, HKV * HD], BF16, name="vtok")
            yt_hi = persist.tile([128, HQ, T], FP8, name="ythi")
            yt_lo = persist.tile([128, HQ, T], FP8, name="ytlo")

            state = {"pending_fin": None}

            def finalize(h, qj, ps_y, ps_d):
                # engines chosen to keep Pool free for the masks (which gate
                # the AV triangle matmuls of the short strip 0): copy on ACT,
                # subtract on DVE
                qsl = bass.ts(qj, 512)
                r_sb = rb.tile([128, 512], F32, name="r")
                nc.vector.reciprocal(r_sb[:], ps_d[:])
                tmp = ytp.tile([128, 512], BF16, name="ytmp")
                nc.vector.tensor_tensor(tmp[:], ps_y[:], r_sb[:], ALU.mult)
                nc.scalar.activation(yt_hi[:, h, qsl], tmp[:], AF.Copy, scale=1.0)
                nc.vector.tensor_tensor(
                    yt_lo[:, h, qsl], tmp[:], yt_hi[:, h, qsl], ALU.subtract
                )

            def make_pending(h, qj, acc, ps_y):
                def flush():
                    ps_d = psD.tile([128, 512], F32, name="psd")
                    nc.tensor.matmul(
                        ps_d[:], ones_mat[:], acc[:], start=True, stop=True
                    )
                    finalize(h, qj, ps_y, ps_d)
                return flush

            def flush_pending():
                if state["pending_fin"] is not None:
                    f = state["pending_fin"]
                    state["pending_fin"] = None
                    f()

            def strip_tiles(h, qj, res, flush_cb):
                """Generator: attention for (h, qj), yielding once per k-tile.

                AV matmuls run four k-tiles behind QK/exp (lag-4 software
                pipeline).  All e-tiles are tree-summed into a single fp16
                accumulator `acc` (full tiles in groups of 4; diagonal tiles
                over their valid column ranges); the single denominator
                ones-matmul is deferred to the next strip (make_pending).
                res[qj] = (ps_y, acc) on completion.
                """
                kvh = h // NREP
                vs = lambda kt: v_sb[:, kt, kvh * HD : (kvh + 1) * HD]
                ps_y = psY.tile([128, 512], F32, name="psy")
                nkt = 4 * qj + 4
                acc = None
                e_d0 = None
                g0 = ga = g2 = None
                hist = []

                def emit_av(e, d, lo, kt):
                    first = kt == 0
                    last = kt == nkt - 1
                    mid = lo + 128
                    if d >= 0 and mid < 512:
                        # valid columns first (no mask dependency), then the
                        # masked 128-wide triangle block
                        nc.tensor.matmul(
                            ps_y[:, mid:512], vs(kt), e[:, mid:512],
                            start=first, stop=False,
                        )
                        nc.tensor.matmul(
                            ps_y[:, lo:mid], vs(kt), e[:, lo:mid],
                            start=False, stop=last,
                        )
                    else:
                        nc.tensor.matmul(
                            ps_y[:, lo:512], vs(kt), e[:, lo:512],
                            start=first, stop=last,
                        )

                for kt in range(nkt):
                    d = kt - 4 * qj
                    # diagonal tile d has valid q-columns only in [128d, 512)
                    lo = 128 * d if d > 0 else 0
                    qlo = qj * 512 + lo
                    ps_s = psS.tile([128, 512], F32, name="pss")
                    nc.tensor.matmul(
                        ps_s[:, lo:512],
                        krot[kvh][:, kt * 128 : (kt + 1) * 128],
                        qrot[h][:, qlo : (qj + 1) * 512],
                        start=True,
                        stop=True,
                    )
                    e = eb.tile([128, 512], BF16, name="e")
                    nc.scalar.activation(
                        e[:, lo:512], ps_s[:, lo:512], AF.Exp, scale=SCALE
                    )
                    if d >= 0:
                        # mask only the 128x128 triangle block, then fold the
                        # valid range into acc.  GPSIMD normally; on head 7
                        # the gpsimd outT store generation would queue ahead
                        # of the masks, and DVE is rope-free there.
                        mask_eng = nc.vector if h == HQ - 1 else nc.gpsimd
                        mask_eng.tensor_tensor(
                            e[:, lo : lo + 128], e[:, lo : lo + 128],
                            mask_sb[:], ALU.mult,
                        )
                        if qj == 0 and d == 0:
                            acc = accp.tile([128, 512], FP16, name="acc")
                            e_d0 = e
                            nc.gpsimd.tensor_copy(acc[:, 0:128], e[:, 0:128])
                        elif qj == 0 and d == 1:
                            nc.vector.tensor_tensor(
                                acc[:, 128:512], e_d0[:, 128:512],
                                e[:, 128:512], ALU.add,
                            )
                        elif d == 0:
                            nc.vector.tensor_tensor(acc[:], acc[:], e[:], ALU.add)
                        else:
                            nc.vector.tensor_tensor(
                                acc[:, lo:512], acc[:, lo:512],
                                e[:, lo:512], ALU.add,
                            )
                    else:
                        # full tiles: tree-sum groups of 4 (first add on
                        # GPSIMD, rest on DVE), merge into acc
                        ph = kt % 4
                        if ph == 0:
                            g0 = e
                        elif ph == 1:
                            ga = gag.tile([128, 512], BF16, name="ga")
                            nc.gpsimd.tensor_tensor(ga[:], g0[:], e[:], ALU.add)
                        elif ph == 2:
                            g2 = e
                        else:
                            if kt // 4 == 0:
                                acc = accp.tile([128, 512], FP16, name="acc")
                                nc.vector.tensor_tensor(
                                    acc[:], g2[:], e[:], ALU.add
                                )
                                nc.vector.tensor_tensor(
                                    acc[:], acc[:], ga[:], ALU.add
                                )
                            else:
                                gs = gag.tile([128, 512], BF16, name="gs")
                                nc.vector.tensor_tensor(gs[:], g2[:], e[:], ALU.add)
                                nc.vector.tensor_tensor(gs[:], gs[:], ga[:], ALU.add)
                                nc.vector.tensor_tensor(
                                    acc[:], acc[:], gs[:], ALU.add
                                )
                    if kt == 2:
                        # previous strip's denominator matmul + finalize slot
                        flush_cb()
                    hist.append((e, d, lo, kt))
                    if len(hist) > 4:
                        emit_av(*hist.pop(0))
                    yield
                for item in hist:
                    emit_av(*item)
                res[qj] = (ps_y, acc)

            def emit_qj(h, qj, pop, flush_cb):
                """Attention for (h, qj), with pop() called once per k-tile."""
                res = {}
                gen = strip_tiles(h, qj, res, flush_cb)
                kt = 0
                while next(gen, _END) is not _END:
                    pop(qj, kt)
                    kt += 1
                return res[qj]

            def emit_pair(h, qa, qb, pop, flush_cb):
                """Attention for strips qa and qb zipped k-tile by k-tile.

                Used for head 7's first two strips, which otherwise have no
                interleave work and run at ACT's exp pace.
                """
                res = {}
                ga = strip_tiles(h, qa, res, flush_cb)
                gb = strip_tiles(h, qb, res, lambda: None)
                kt = 0
                while True:
                    a_end = next(ga, _END) is _END
                    b_end = next(gb, _END) is _END
                    if a_end and b_end:
                        break
                    pop(qa, kt)
                    kt += 1
                return res[qa], res[qb]

            _END = object()

            # ======== projection machinery (lives through heads 0..6) ========
            with (
                tc.tile_pool(name="xa", bufs=1) as xa,
                tc.tile_pool(name="wm", bufs=4) as wm,
                tc.tile_pool(name="ta", bufs=1) as ta,
                tc.tile_pool(name="psA", bufs=2, space="PSUM") as psA,
                tc.tile_pool(name="psP2", bufs=1, space="PSUM") as psP2,
            ):
                xs_hi = xa.tile([128, KC, T], FP8, name="xshi")
                xs_lo = xa.tile([128, KC, T], FP8, name="xslo")

                def load_wm(m, split=False):
                    w = wm.tile([128, 2, KC, 128], FP8, name="wmhl")
                    src = wqk_hl[m].rearrange("p (l kc c) -> p l kc c", l=2, kc=KC)
                    if split:
                        nc.sync.dma_start(w[:, 0], src[:, 0])
                        nc.sync.dma_start(w[:, 1], src[:, 1])
                    else:
                        nc.sync.dma_start(w[:], src)
                    return w

                def proj_mms(ps, w, nsl):
                    """Triple-MM qkv projection chunk stream for one strip."""
                    for p in range(KC // 2):
                        sl = slice(2 * p, 2 * p + 2)
                        nc.tensor.matmul(
                            ps[:], w[:, 0, sl, :], xs_hi[:, sl, nsl],
                            start=(p == 0), stop=False, perf_mode=DR,
                        )
                        nc.tensor.matmul(
                            ps[:], w[:, 0, sl, :], xs_lo[:, sl, nsl],
                            start=False, stop=False, perf_mode=DR,
                        )
                        nc.tensor.matmul(
                            ps[:], w[:, 1, sl, :], xs_hi[:, sl, nsl],
                            start=False, stop=(p == KC // 2 - 1), perf_mode=DR,
                        )
                        yield

                def rope_ops(m, n, ps):
                    """The four RoPE ops for one (feature tile, strip) pair."""
                    dst = qrot[m] if m < HQ else krot[m - HQ]
                    nsl = bass.ts(n, 512)
                    pt = ta.tile([128, 512], F32, name="pt")
                    p2 = psP2.tile([128, 512], F32, name="p2")
                    yield nc.vector.tensor_tensor(
                        pt[:], ps[:], trigf_sb[:, nsl], ALU.mult
                    )
                    yield nc.vector.tensor_tensor(
                        p2[:], ps[:], trigw_sb[:, nsl], ALU.mult
                    )
                    yield nc.vector.tensor_tensor(
                        dst[0:64, nsl], pt[0:64, :], p2[64:128, :], ALU.subtract
                    )
                    yield nc.vector.tensor_tensor(
                        dst[64:128, nsl], p2[0:64, :], pt[64:128, :], ALU.add
                    )

                wnext = {}

                def a_stream(m, pool):
                    w = wnext.pop(m)
                    yield
                    for n in range(NQ):
                        nsl = bass.ts(n, 512)
                        ps = pool.tile([128, 512], F32, name="psA")
                        yield from proj_mms(ps[:], w, nsl)
                        if n == 2 and m + 1 < HQ:
                            # prefetch the NEXT head's weights mid-stream so
                            # the first chunk of the next a_stream never waits
                            # on a cold DMA at the head boundary
                            wnext[m + 1] = load_wm(m + 1)
                        for _ in rope_ops(m, n, ps):
                            yield

                # ---- A0: v projection + k heads + q head 0 (pure PE phase) ----
                with tc.tile_pool(name="wvp", bufs=1) as wvp:
                    wv_sbh = wvp.tile([128, KC, HKV * HD], FP8, name="wvh")
                    wv_sbl = wvp.tile([128, KC, HKV * HD], FP8, name="wvl")
                    # startup DMA order matters: the first proj accumulation
                    # group needs x kc 0:2 (hi+lo) + all of wk0; wk1/wq0 come
                    # right after because strip 0 runs chunk-major across all
                    # three m-streams (transfers serialize on the DMA engines,
                    # ~720ns per 0.25 MB).
                    wk0 = load_wm(HQ)
                    nc.sync.dma_start(xs_hi[:, 0:2, 0:512], xhi[:, 0:2, 0:512])
                    nc.sync.dma_start(xs_lo[:, 0:2, 0:512], xlo[:, 0:2, 0:512])
                    wk1 = load_wm(HQ + 1)
                    wq0 = load_wm(0)
                    nc.sync.dma_start(xs_hi[:, 2:6, 0:512], xhi[:, 2:6, 0:512])
                    nc.sync.dma_start(xs_lo[:, 2:6, 0:512], xlo[:, 2:6, 0:512])
                    nc.sync.dma_start(xs_hi[:, 6:10, 0:512], xhi[:, 6:10, 0:512])
                    nc.sync.dma_start(xs_lo[:, 6:10, 0:512], xlo[:, 6:10, 0:512])
                    nc.sync.dma_start(trigf_sb[:, 0:512], trigf[:, 0:512])
                    nc.sync.dma_start(trigw_sb[:, 0:512], trigw[:, 0:512])
                    nc.sync.dma_start(xs_hi[:, 10:14, 0:512], xhi[:, 10:14, 0:512])
                    nc.sync.dma_start(xs_lo[:, 10:14, 0:512], xlo[:, 10:14, 0:512])
                    nc.sync.dma_start(xs_hi[:, 14:16, 0:512], xhi[:, 14:16, 0:512])
                    nc.sync.dma_start(xs_lo[:, 14:16, 0:512], xlo[:, 14:16, 0:512])
                    wvh4 = wv_hi.rearrange("p (kc c) -> p kc c", kc=KC)
                    wvl4 = wv_lo.rearrange("p (kc c) -> p kc c", kc=KC)
                    nc.sync.dma_start(wv_sbh[:, 0:8], wvh4[:, 0:8])
                    nc.sync.dma_start(wv_sbl[:, 0:8], wvl4[:, 0:8])
                    nc.sync.dma_start(wv_sbh[:, 8:16], wvh4[:, 8:16])
                    nc.sync.dma_start(wv_sbl[:, 8:16], wvl4[:, 8:16])
                    nc.sync.dma_start(mask_sb[:], maskd[:])
                    nc.vector.memset(ones_mat[:], 1.0)

                    def v_proj_strip(n):
                        for tt in range(4 * n, 4 * n + 4):
                            # reuse the attention-phase psum slots during A0
                            tsl = slice(tt * 128, (tt + 1) * 128)
                            psv = psS.tile([128, 512], F32, name="pss")[
                                :, : HKV * HD
                            ]
                            for p in range(KC // 2):
                                sl = slice(2 * p, 2 * p + 2)
                                nc.tensor.matmul(
                                    psv[:], xs_hi[:, sl, tsl], wv_sbh[:, sl, :],
                                    start=(p == 0), stop=False, perf_mode=DR,
                                )
                                nc.tensor.matmul(
                                    psv[:], xs_hi[:, sl, tsl], wv_sbl[:, sl, :],
                                    start=False, stop=False, perf_mode=DR,
                                )
                                nc.tensor.matmul(
                                    psv[:], xs_lo[:, sl, tsl], wv_sbh[:, sl, :],
                                    start=False, stop=(p == KC // 2 - 1),
                                    perf_mode=DR,
                                )
                            nc.scalar.activation(
                                v_sb[:, tt, :], psv[:], AF.Copy, scale=1.0 / WS
                            )

                    for n in range(NQ):
                        nsl = bass.ts(n, 512)
                        # chunk-major across the three m-streams: the early
                        # strips are DMA-paced, so burn each arriving x chunk
                        # three times before needing the next
                        accs = [
                            (HQ, wk0, psY.tile([128, 512], F32, name="psy")),
                            (HQ + 1, wk1, psY.tile([128, 512], F32, name="psy")),
                            (0, wq0, psD.tile([128, 512], F32, name="psd")),
                        ]
                        gens = [
                            (m, proj_mms(ps[:], w, nsl)) for m, w, ps in accs
                        ]
                        for p in range(KC // 2):
                            for m, g in gens:
                                next(g)
                            if p == 2 and n + 1 < NQ:
                                # next-strip loads overlap this strip's tail;
                                # strip 1 is still DMA-paced, so use finer
                                # chunks there to avoid a late lo-half
                                nsl_next = bass.ts(n + 1, 512)
                                ksls = (
                                    [slice(4 * i, 4 * i + 4) for i in range(4)]
                                    if n == 0
                                    else [slice(0, 8), slice(8, 16)]
                                )
                                for ksl in ksls:
                                    nc.sync.dma_start(
                                        xs_hi[:, ksl, nsl_next],
                                        xhi[:, ksl, nsl_next],
                                    )
                                    nc.sync.dma_start(
                                        xs_lo[:, ksl, nsl_next],
                                        xlo[:, ksl, nsl_next],
                                    )
                                nc.sync.dma_start(
                                    trigf_sb[:, nsl_next], trigf[:, nsl_next]
                                )
                                nc.sync.dma_start(
                                    trigw_sb[:, nsl_next], trigw[:, nsl_next]
                                )
                        for m, w, ps in accs:
                            for _ in rope_ops(m, n, ps):
                                pass
                        # v projection runs one strip late so its weight DMAs
                        # hide behind the first two m-stream batches
                        if n >= 1:
                            v_proj_strip(n - 1)
                    wnext[1] = load_wm(1)
                    v_proj_strip(NQ - 2)
                    v_proj_strip(NQ - 1)

                # ---- heads 0..6: attention + next head's projection ----
                for h in range(HQ - 1):
                    agen = a_stream(h + 1, psA)

                    def pop(qj, kt, agen=agen):
                        # 49 stream items vs 40 k-tiles: the extra pulls go
                        # mid-strip, and the stream finishes by mid strip 3 so
                        # its final rope chain (which holds the psA bank) is
                        # long drained before the next head's stream starts
                        next(agen, None)
                        if 4 <= kt <= 6 or (qj == 3 and 7 <= kt <= 9):
                            next(agen, None)

                    for qj in range(NQ):
                        ps_y, acc = emit_qj(h, qj, pop, flush_pending)
                        state["pending_fin"] = make_pending(h, qj, acc, ps_y)
                    for _ in agen:
                        pass

            # ---- head 7: attention + output projection interleaved ----
            with (
                tc.tile_pool(name="wpc", bufs=12) as wpc,
                tc.tile_pool(name="obp", bufs=4) as obp,
                tc.tile_pool(name="psO", bufs=3, space="PSUM") as psO,
            ):
                def load_wp(fm):
                    wc = wpc.tile([128, 2, HQ, 128], FP8, name="wc")
                    nc.sync.dma_start(
                        wc[:], wp_hl[fm].rearrange("p (l h c) -> p l h c", l=2, h=HQ)
                    )
                    return wc

                def c_stream(n, wcs, last=False):
                    """Output projection for token strip n (16 feature tiles)."""
                    nsl = bass.ts(n, 512)
                    for fm in range(FM):
                        wc = wcs.pop(0)
                        yield
                        fsl = slice(fm * 128, (fm + 1) * 128)
                        ps_o = psO.tile([128, 512], F32, name="pso")
                        if last and fm == FM - 1:
                            # the very last tile: two independent column-group
                            # accumulations, so the first half's output drains
                            # while the second half is still on the PE
                            for half in range(2):
                                csl = slice(256 * half, 256 * half + 256)
                                qsl = slice(n * 512 + 256 * half,
                                            n * 512 + 256 * half + 256)
                                for p in range(HQ // 2):
                                    sl = slice(2 * p, 2 * p + 2)
                                    nc.tensor.matmul(
                                        ps_o[:, csl], wc[:, 0, sl, :],
                                        yt_hi[:, sl, qsl],
                                        start=(p == 0), stop=False, perf_mode=DR,
                                    )
                                    nc.tensor.matmul(
                                        ps_o[:, csl], wc[:, 0, sl, :],
                                        yt_lo[:, sl, qsl],
                                        start=False, stop=False, perf_mode=DR,
                                    )
                                    nc.tensor.matmul(
                                        ps_o[:, csl], wc[:, 1, sl, :],
                                        yt_hi[:, sl, qsl],
                                        start=False, stop=(p == HQ // 2 - 1),
                                        perf_mode=DR,
                                    )
                                    yield
                                ob = obp.tile([128, 256], BF16, name="obh")
                                nc.scalar.activation(
                                    ob[:], ps_o[:, csl], AF.Copy, scale=1.0 / WS
                                )
                                eng = nc.gpsimd if half == 0 else nc.sync
                                eng.dma_start(outT[fsl, qsl], ob[:])
                            yield
                            continue
                        for p in range(HQ // 2):
                            sl = slice(2 * p, 2 * p + 2)
                            nc.tensor.matmul(
                                ps_o[:], wc[:, 0, sl, :], yt_hi[:, sl, nsl],
                                start=(p == 0), stop=False, perf_mode=DR,
                            )
                            nc.tensor.matmul(
                                ps_o[:], wc[:, 0, sl, :], yt_lo[:, sl, nsl],
                                start=False, stop=False, perf_mode=DR,
                            )
                            nc.tensor.matmul(
                                ps_o[:], wc[:, 1, sl, :], yt_hi[:, sl, nsl],
                                start=False, stop=(p == HQ // 2 - 1),
                                perf_mode=DR,
                            )
                            if p == 0 and fm + 3 < FM:
                                wcs.append(load_wp(fm + 3))
                            yield
                        ob = obp.tile([128, 512], BF16, name="ob")
                        nc.scalar.activation(ob[:], ps_o[:], AF.Copy, scale=1.0 / WS)
                        # stores alternate gpsimd/SP (both mostly idle here):
                        # a single engine's ~1us per-store issue would
                        # serialize the drain tail, and on ACT it delays the
                        # ob copies that free psO
                        eng = nc.gpsimd if fm % 2 == 0 else nc.sync
                        eng.dma_start(outT[fsl, nsl], ob[:])
                        yield

                cgens = []

                _end = object()

                def pull(n):
                    for _ in range(n):
                        while cgens:
                            if next(cgens[0], _end) is _end:
                                cgens.pop(0)
                            else:
                                break

                def pop7(qj, kt):
                    pull(7)

                # head 6's strip-3 denominator must flush before the pair:
                # psY only holds two live accumulators
                flush_pending()
                wcs0 = [load_wp(0), load_wp(1), load_wp(2)]
                wcs1 = [load_wp(0), load_wp(1), load_wp(2)]
                (ps_y1, acc1), (ps_y0, acc0) = emit_pair(
                    HQ - 1, 1, 0, pop7, flush_pending
                )
                make_pending(HQ - 1, 0, acc0, ps_y0)()
                cgens.append(c_stream(0, wcs0))
                make_pending(HQ - 1, 1, acc1, ps_y1)()
                cgens.append(c_stream(1, wcs1))
                for qj in range(2, NQ):
                    # issue the strip's first wp loads before its attention so
                    # the c_proj stream never starts on a cold DMA
                    wcs = [load_wp(0), load_wp(1), load_wp(2)]
                    ps_y, acc = emit_qj(HQ - 1, qj, pop7, flush_pending)
                    # cover the strip's sum-chain tail with c_proj work from
                    # the previous strip before emitting its denominator
                    pull(16)
                    make_pending(HQ - 1, qj, acc, ps_y)()
                    cgens.append(c_stream(qj, wcs, last=False))
                # drain remaining output projection
                for g in cgens:
                    for _ in g:
                        pass

    nc.compile()
    return nc


def _get_nc():
    global _NC
    if _NC is None:
        _NC = build_nc()
    return _NC


def _split_hilo(a):
    """a = hi + lo with both parts e4m3."""
    hi = a.astype(E4)
    lo = (a - hi.astype(np.float32)).astype(E4)
    return hi, lo


def _prep_inputs(x, w_attn, w_proj):
    """Build the 8 per-core input maps from the full-problem arrays."""
    perm = np.concatenate([np.arange(0, HD, 2), np.arange(1, HD, 2)])

    f = np.arange(64, dtype=np.float64)
    inv = ROPE_THETA ** (-2.0 * f / HD)
    ang = inv[:, None] * np.arange(T, dtype=np.float64)[None, :]
    trigc = (np.cos(ang) / WS).astype(BF)
    trigs = (np.sin(ang) / WS).astype(BF)
    trigf = np.ascontiguousarray(np.concatenate([trigc, trigc], axis=0))
    trigw = np.ascontiguousarray(np.concatenate([trigs, trigs], axis=0))

    kk = np.arange(128)[:, None]
    qq = np.arange(128)[None, :]
    maskd = (kk <= qq).astype(BF)  # [128 k, 128 q] lower-triangle-valid

    w_attn = np.asarray(w_attn)
    w_proj = np.asarray(w_proj)
    x = np.asarray(x)

    in_maps = []
    for core in range(N_CORES):
        b, g = core // TP, core % TP
        # x features chunked: xhi[p, kc, t] = x[b].T[kc*128+p, t]
        xT = np.ascontiguousarray(x[b].T)  # [C, T] f32
        x_hi, x_lo = _split_hilo(xT)
        xhi = np.ascontiguousarray(x_hi.reshape(KC, 128, T).transpose(1, 0, 2))
        xlo = np.ascontiguousarray(x_lo.reshape(KC, 128, T).transpose(1, 0, 2))

        qrows = []
        for h in range(HQ):
            gh = g * HQ + h
            qrows.append(gh * HD + perm)
        for kv in range(HKV):
            gk = g * HKV + kv
            qrows.append(N_HEAD * HD + gk * HD + perm)
        qrows = np.concatenate(qrows)
        wqk = w_attn[qrows] * WS  # [1280, C] f32
        # stationary layout: wqk3[m, p, kc*128+col] = wqk[m*128+col, kc*128+p]
        wqk3 = np.ascontiguousarray(
            wqk.reshape(MQK, 128, KC, 128).transpose(0, 3, 2, 1).reshape(MQK, 128, C)
        )
        wqk3_hi, wqk3_lo = _split_hilo(wqk3)
        wqk_hl = np.ascontiguousarray(
            np.stack([wqk3_hi, wqk3_lo], axis=2).reshape(MQK, 128, 2 * C)
        )

        vrows = np.concatenate(
            [
                (N_HEAD + N_KV_HEAD) * HD + (g * HKV + kv) * HD + np.arange(HD)
                for kv in range(HKV)
            ]
        )
        wv = w_attn[vrows] * WS  # [256, C]
        # wv3[p, kc*256+c] = wv[c, kc*128+p]
        wv3 = np.ascontiguousarray(
            wv.reshape(HKV * HD, KC, 128).transpose(2, 1, 0).reshape(128, KC * HKV * HD)
        )
        wv3_hi, wv3_lo = _split_hilo(wv3)

        cols = np.arange(g * HQ * HD, (g + 1) * HQ * HD)
        wpg = w_proj[:, cols] * WS  # [C, 1024], rows = out features
        # wp5[fm, d, h*128+p] = wpg[fm*128+p, h*128+d]
        wp5 = np.ascontiguousarray(
            wpg.T.reshape(HQ, 128, FM, 128).transpose(2, 1, 0, 3).reshape(FM, 128, HQ * 128)
        )
        wp5_hi, wp5_lo = _split_hilo(wp5)
        wp_hl = np.ascontiguousarray(
            np.stack([wp5_hi, wp5_lo], axis=2).reshape(FM, 128, 2 * HQ * 128)
        )

        in_maps.append(
            {
                "xhi": xhi,
                "xlo": xlo,
                "wqk_hl": wqk_hl,
                "wv_hi": wv3_hi,
                "wv_lo": wv3_lo,
                "wp_hl": wp_hl,
                "trigf": trigf,
                "trigw": trigw,
                "maskd": maskd,
            }
        )
    return in_maps


def kernel(x, w_attn, w_proj):
    global LAST_RUN
    nc = _get_nc()
    in_maps = _prep_inputs(x, w_attn, w_proj)
    res = run_bass_kernel_spmd(nc, in_maps, core_ids=list(range(N_CORES)))
    LAST_RUN = res
    out = np.empty((B, T, C), dtype=np.float32)
    for b in range(B):
        acc = (
            res.results[TP * b]["outT"].astype(np.float32)
            + res.results[TP * b + 1]["outT"].astype(np.float32)
        )
        out[b] = acc.T
    return out
